# revision 8
# baseline (speedup 1.0000x reference)
"""CascadeHadamardLinear Trainium2 kernel (8-core data-parallel over tokens).

Math per token row x[4096]:
  x_rot = (x * S_in) @ blockdiag(H_128)
  x_q   = NVFP4 fake-quant of x_rot (16-elem blocks, e2m1 snap, RNE)
  out   = x_q @ W^T + (x_rot @ la^T) @ lb^T + bias

Host side: shard 8192 tokens over 8 cores (1024 each). x is shipped as
bf16; the Hadamard is shipped as the exact-in-bf16 sign matrix
(H*sqrt(128))*S folded per block, so the rotation matmuls run in bf16
with the 1/sqrt(128) constant folded into the quant scale. la_eff
(diag(S) Hbd la^T) is precomputed on host. lbT carries a 33rd row =
bias, matched by a ones-row in t1T, so bias needs no separate matmul.

Device, per core, tokens processed in 4 quarters of 256:
  P1(q): per 4-block group jg: DMA xt [128,4,256] bf16; LoRA1 t1 acc
    (PSUM, K=128 per j); rotation MMs -> PSUM bank [128t, 512] fp32;
    quant: absmax-16 (DVE), recip (DVE), z = x*(6/amax) (DVE TT),
    2 fused custom DVE snap ops (branch-free e2m1 via 1.5*2^k magic-add
    + selects on z^2), xq = f*(amax*h/6) -> bf16 (GPSIMD); PE-transpose
    xq -> pt PSUM -> xqT (ACT copy).
  P2: og-outer (W loaded once), th-inner over all 8 token tiles:
    out[t,o] PSUM chain: 32 xqT MMs + t1T^T @ lbT_ext (includes bias),
    ACT evac bf16, DMA out. Scheduler backfills PE idle during P1 with
    ready P2 matmuls (quarter q unlocks th=2q,2q+1 for all og).
"""

import os
import sys

for _p in ("/opt/trn_rl_repo",):
    if os.path.isdir(_p) and _p not in sys.path:
        sys.path.insert(0, _p)

import numpy as np

import concourse.bass as bass
import concourse.mybir as mybir
import concourse.tile as tile
from concourse import bacc
from concourse.bass_utils import run_bass_kernel_spmd

F32 = np.float32

# ---------------- problem constants (hardcoded per contract) ----------------
B, S, D_IN, D_OUT, RANK, HBS = 4, 2048, 4096, 4096, 32, 128
NTOK = B * S                  # 8192
NCORES = 8
NT = NTOK // NCORES           # 1024 tokens per core
NJ = D_IN // HBS              # 32 hadamard blocks
QB = 16                       # quant block size
NQ = 4                        # token quarters per core
QT = NT // NQ                 # 256 tokens per quarter
OG_N = D_OUT // 512           # 8 output column groups

# 1/sqrt(128) exactly as the reference's fp32 H entries have it
H_SCALE = float(np.float32(np.float64(1.0) / np.sqrt(np.float64(128.0))))

# quant snap constants (1.5*2^k magic so ulp is uniform on both sides of c)
C_INT = 12582912.0            # 1.5*2^23, ulp 1.0
C_HALF = 6291456.0            # 1.5*2^22, ulp 0.5
TH23 = 20.25                  # 4.5^2
THF = 5.0625                  # 2.25^2

# ---------------- custom DVE ops (e2m1 level snap) ----------------
def _register_snap_ops():
    from concourse.dve_spec import (
        Spec, Src0, Src1, C0, C1, C2, lower as dve_lower, sq, select, _has_src1,
    )
    from concourse.dve_ops import (
        DveOp, OPS, CUSTOM_DVE_SPECS, _SUB_OPCODE_FOR_NAME, _CUSTOM_DVE_ROW_BASE,
    )
    from concourse.dve_uop import DveOpSpec
    from concourse.dve_table_gen import dve_ver_for

    def _ref_midlow(in0, in1, c0, c1, c2):
        z = in0.astype(F32)
        c1 = F32(c1)
        c2 = F32(c2)
        th = (z + c1) - c1
        ti = (z + c2) - c2
        return np.where(z * z < F32(c0), th, ti).astype(F32)

    def _ref_sel23(in0, in1, c0, c1, c2):
        z = in0.astype(F32)
        c3 = F32(2.0) * F32(c1)
        t3 = (z + c3) - c3
        return np.where(z * z < F32(c0), in1.astype(F32), t3).astype(F32)

    def _mk(name, body, ref):
        if name in _SUB_OPCODE_FOR_NAME:
            return next(op for op in OPS if op.name == name)
        spec = Spec(body=body, reference=ref)
        row = _CUSTOM_DVE_ROW_BASE + len(OPS)
        assert row < 0x20
        ver = dve_ver_for("TRN2")
        uops = dve_lower(spec, ver=ver)
        sha = DveOpSpec(
            name=name, opcode=row, uops=uops, rd1_en=_has_src1(spec)
        ).sha(ver)
        op = DveOp(name, spec, subdim=False, uops_sha={ver: sha})
        OPS.append(op)
        CUSTOM_DVE_SPECS[name] = spec
        _SUB_OPCODE_FOR_NAME[name] = row
        return op

    z = Src0
    midlow = _mk(
        "SNAP_MIDLOW_ANT",
        select(sq(z) < C0, (z + C1) - C1, (z + C2) - C2),
        _ref_midlow,
    )
    c3 = C1 + C1
    sel23 = _mk(
        "SNAP_SEL23_ANT",
        select(sq(z) < C0, Src1, (z + c3) - c3),
        _ref_sel23,
    )
    return midlow, sel23


SNAP_MIDLOW, SNAP_SEL23 = _register_snap_ops()


# ---------------- device kernel ----------------
def _build_nc():
    nc = bacc.Bacc(
        "TRN2", target_bir_lowering=False, debug=False, num_devices=NCORES
    )
    dt = mybir.dt
    xTh = nc.dram_tensor("xTh", [D_IN, NT], dt.bfloat16, kind="ExternalInput")
    xTl = nc.dram_tensor("xTl", [D_IN, NT], dt.bfloat16, kind="ExternalInput")
    wT = nc.dram_tensor("wT", [D_IN, D_OUT], dt.bfloat16, kind="ExternalInput")
    HmS = nc.dram_tensor("HmS", [HBS, NJ, HBS], dt.bfloat16, kind="ExternalInput")
    laE = nc.dram_tensor("laE", [HBS, NJ, RANK], dt.bfloat16, kind="ExternalInput")
    lbTe = nc.dram_tensor("lbTe", [RANK + 1, D_OUT], dt.bfloat16, kind="ExternalInput")
    ident = nc.dram_tensor("ident", [128, 128], dt.bfloat16, kind="ExternalInput")
    y = nc.dram_tensor("y", [NT, D_OUT], dt.bfloat16, kind="ExternalOutput")

    with tile.TileContext(nc) as tc:
        _emit(nc, tc, xTh, xTl, wT, HmS, laE, lbTe, ident, y)
    nc.compile()
    return nc


def _emit(nc, tc, xTh, xTl, wT, HmS, laE, lbTe, ident, y):
    from contextlib import ExitStack

    dt = mybir.dt
    Alu = mybir.AluOpType

    with ExitStack() as ctx:
        consts = ctx.enter_context(tc.tile_pool(name="consts", bufs=1))
        xqT_pool = ctx.enter_context(tc.tile_pool(name="xqT", bufs=1))

        H_sb = consts.tile([HBS, NJ, HBS], dt.bfloat16)
        nc.sync.dma_start(out=H_sb[:], in_=HmS[:])
        la_sb = consts.tile([HBS, NJ, RANK], dt.bfloat16)
        nc.sync.dma_start(out=la_sb[:], in_=laE[:])
        lb_sb = consts.tile([RANK + 1, D_OUT], dt.bfloat16)
        nc.sync.dma_start(out=lb_sb[:], in_=lbTe[:])
        id_sb = consts.tile([128, 128], dt.bfloat16)
        nc.sync.dma_start(out=id_sb[:], in_=ident[:])
        t1T = consts.tile([RANK + 1, NT], dt.bfloat16)
        nc.vector.memset(t1T[RANK : RANK + 1, :], 1.0)

        # xqT[c, j, t] : feature-major quantized activations (bf16)
        xqT = xqT_pool.tile([128, NJ, NT], dt.bfloat16)

        xt_pool = ctx.enter_context(tc.tile_pool(name="xt", bufs=3))
        xtl_pool = ctx.enter_context(tc.tile_pool(name="xtl", bufs=3))
        qsm = ctx.enter_context(tc.tile_pool(name="qsm", bufs=8))
        qtmp = ctx.enter_context(tc.tile_pool(name="qtmp", bufs=6))
        xq_pool = ctx.enter_context(tc.tile_pool(name="xq", bufs=4))
        wbf_pool = ctx.enter_context(tc.tile_pool(name="wbf", bufs=2))
        out_pool = ctx.enter_context(tc.tile_pool(name="out", bufs=4))
        rot_ps = ctx.enter_context(tc.tile_pool(name="rotps", bufs=2, space="PSUM"))
        pt_ps = ctx.enter_context(tc.tile_pool(name="ptps", bufs=2, space="PSUM"))
        t1_ps = ctx.enter_context(tc.tile_pool(name="t1ps", bufs=2, space="PSUM"))
        out_ps = ctx.enter_context(tc.tile_pool(name="outps", bufs=2, space="PSUM"))

        def emit_p1(q):
            qsl = slice(q * QT, (q + 1) * QT)
            t1a = t1_ps.tile([RANK, QT], dt.float32, name=f"t1a{q}", tag="t1a")
            for jg in range(NJ // 4):
                xth = xt_pool.tile([128, 4, QT], dt.bfloat16, name=f"xth{q}_{jg}", tag="xth")
                nc.sync.dma_start(
                    out=xth[:],
                    in_=xTh[jg * 512 : (jg + 1) * 512, qsl].rearrange(
                        "(j c) t -> c j t", c=HBS
                    ),
                )
                xtl = xtl_pool.tile([128, 4, QT], dt.bfloat16, name=f"xtl{q}_{jg}", tag="xtl")
                nc.sync.dma_start(
                    out=xtl[:],
                    in_=xTl[jg * 512 : (jg + 1) * 512, qsl].rearrange(
                        "(j c) t -> c j t", c=HBS
                    ),
                )
                for dj in range(4):
                    j = 4 * jg + dj
                    nc.tensor.matmul(
                        t1a[:], lhsT=la_sb[:, j, :], rhs=xth[:, dj, :],
                        start=(j == 0), stop=(j == NJ - 1),
                    )
                xq_tiles = []
                for ts in range(QT // 128):
                    bank = rot_ps.tile([128, 512], dt.float32,
                                       name=f"bank{q}{jg}{ts}", tag="bank")
                    for dj in range(4):
                        j = 4 * jg + dj
                        for hl, xt_ in enumerate((xth, xtl)):
                            nc.tensor.matmul(
                                bank[:, dj * HBS : (dj + 1) * HBS],
                                lhsT=xt_[:, dj, ts * 128 : (ts + 1) * 128],
                                rhs=H_sb[:, j, :],
                                start=(dj == 0 and hl == 0),
                                stop=(dj == 3 and hl == 1),
                            )
                    nb = 512 // QB
                    amax = qsm.tile([128, nb], dt.float32, name=f"amax{q}{jg}{ts}", tag="amax")
                    nc.vector.tensor_reduce(
                        out=amax[:], in_=bank[:].rearrange("p (b s) -> p b s", s=QB),
                        axis=mybir.AxisListType.X, op=Alu.max,
                        apply_absolute_value=True,
                    )
                    ra = qsm.tile([128, nb], dt.float32, name=f"ra{q}{jg}{ts}", tag="ra")
                    nc.vector.reciprocal(out=ra[:], in_=amax[:])
                    rs6 = qsm.tile([128, nb], dt.float32, name=f"rs6{q}{jg}{ts}", tag="rs6")
                    nc.vector.tensor_scalar(
                        out=rs6[:], in0=ra[:], scalar1=6.0, scalar2=None,
                        op0=Alu.mult,
                    )
                    sc = qsm.tile([128, nb], dt.float32, name=f"sc{q}{jg}{ts}", tag="sc")
                    nc.scalar.mul(out=sc[:], in_=amax[:], mul=H_SCALE / 6.0)
                    z = qtmp.tile([128, 512], dt.float32, name=f"z{q}{jg}{ts}", tag="qt")
                    nc.vector.tensor_tensor(
                        out=z[:].rearrange("p (b s) -> p b s", s=QB),
                        in0=bank[:].rearrange("p (b s) -> p b s", s=QB),
                        in1=rs6[:].unsqueeze(2).broadcast_to([128, nb, QB]),
                        op=Alu.mult,
                    )
                    r = qtmp.tile([128, 512], dt.float32, name=f"r{q}{jg}{ts}", tag="qt")
                    nc.vector._custom_dve(
                        SNAP_MIDLOW, out=r[:], in0=z[:], s0=THF, s1=C_HALF,
                        imm2=C_INT,
                    )
                    f = qtmp.tile([128, 512], dt.float32, name=f"f{q}{jg}{ts}", tag="qt")
                    nc.vector._custom_dve(
                        SNAP_SEL23, out=f[:], in0=z[:], in1=r[:], s0=TH23, s1=C_INT,
                    )
                    xq_t = xq_pool.tile([128, 512], dt.bfloat16, name=f"xq{q}{jg}{ts}", tag="xq")
                    nc.gpsimd.tensor_tensor(
                        out=xq_t[:].rearrange("p (b s) -> p b s", s=QB),
                        in0=f[:].rearrange("p (b s) -> p b s", s=QB),
                        in1=sc[:].unsqueeze(2).broadcast_to([128, nb, QB]),
                        op=Alu.mult,
                    )
                    xq_tiles.append(xq_t)
                pt = pt_ps.tile([128, 4, QT], dt.bfloat16, name=f"pt{q}_{jg}", tag="pt")
                nts = QT // 128
                for dj in range(4):
                    for ts in range(nts):
                        nc.tensor.matmul(
                            pt[:, dj, ts * 128 : (ts + 1) * 128],
                            lhsT=xq_tiles[ts][:, dj * HBS : (dj + 1) * HBS],
                            rhs=id_sb[:], is_transpose=True,
                            start=(dj == 0 and ts == 0),
                            stop=(dj == 3 and ts == nts - 1),
                        )
                nc.scalar.copy(out=xqT[:, 4 * jg : 4 * jg + 4, qsl], in_=pt[:])
            nc.scalar.copy(out=t1T[0:RANK, qsl], in_=t1a[:])

        def load_wbf(og):
            osl = slice(og * 512, (og + 1) * 512)
            wbf = wbf_pool.tile([128, NJ, 512], dt.bfloat16, name=f"wbf{og}", tag="wbf")
            # scalar (ACT) hwdge queue: decoupled from the xt issue stream on
            # sync, so W flows while P1's xt loads are still slot-blocked
            nc.scalar.dma_start(
                out=wbf[:], in_=wT[:, osl].rearrange("(j c) o -> c j o", c=HBS)
            )
            return wbf

        def emit_p2(og, wbf):
            osl = slice(og * 512, (og + 1) * 512)
            for th in range(NT // 128):
                tsl = slice(th * 128, (th + 1) * 128)
                po = out_ps.tile([128, 512], dt.float32, name=f"po{og}{th}", tag="po")
                for j in range(NJ):
                    nc.tensor.matmul(
                        po[:], lhsT=xqT[:, j, tsl], rhs=wbf[:, j, :],
                        start=(j == 0), stop=False,
                    )
                nc.tensor.matmul(
                    po[:], lhsT=t1T[:, tsl], rhs=lb_sb[:, osl],
                    start=False, stop=True,
                )
                ot = out_pool.tile([128, 512], dt.bfloat16, name=f"ot{og}{th}", tag="ot")
                nc.scalar.copy(out=ot[:], in_=po[:])
                nc.scalar.dma_start(out=y[tsl, osl], in_=ot[:])

        wbf_pre = [load_wbf(0), load_wbf(1)]
        for q in range(NQ):
            emit_p1(q)
        for og in range(OG_N):
            wbf = wbf_pre[og] if og < len(wbf_pre) else load_wbf(og)
            emit_p2(og, wbf)


_NC_CACHE = None


def _get_nc():
    global _NC_CACHE
    if _NC_CACHE is None:
        _NC_CACHE = _build_nc()
    return _NC_CACHE


# ---------------- host wrapper ----------------
def make_in_maps(x, S_in, H_block, w_quantized, lora_a, lora_b, bias):
    import ml_dtypes
    BF16 = ml_dtypes.bfloat16

    x = np.asarray(x, dtype=F32)
    S_in = np.asarray(S_in, dtype=F32)
    H_block = np.asarray(H_block, dtype=F32)
    w_quantized = np.asarray(w_quantized, dtype=F32)
    lora_a = np.asarray(lora_a, dtype=F32)
    lora_b = np.asarray(lora_b, dtype=F32)
    bias = np.asarray(bias, dtype=F32)

    x_flat = x.reshape(NTOK, D_IN)
    wT = np.ascontiguousarray(w_quantized.T.astype(BF16))   # [D_IN, D_OUT]

    # sign matrix: H_block = Hpm * (1/sqrt(128)); Hpm entries are +-1 (bf16 exact)
    Hpm = np.where(H_block > 0, np.float32(1.0), np.float32(-1.0))
    Sc = S_in.reshape(NJ, HBS)                              # [j, r]
    # HmS[r, j, c] = Hpm[r, c] * S[j*128+r]
    HmS = np.ascontiguousarray(
        (Hpm[None, :, :] * Sc[:, :, None]).transpose(1, 0, 2).astype(BF16)
    )
    # la_eff[c, j, r] = S[j*128+c] * sum_k H_block[k, c] * lora_a[r, j*128+k]
    la_blk = lora_a.reshape(RANK, NJ, HBS)                  # [r, j, k]
    la_rot = np.einsum("kc,rjk->cjr", H_block, la_blk)      # [c, j, r]
    laE = np.ascontiguousarray((la_rot * Sc.T[:, :, None]).astype(BF16))
    lbTe = np.ascontiguousarray(
        np.concatenate([lora_b.T, bias.reshape(1, D_OUT)], axis=0).astype(BF16)
    )
    ident = np.ascontiguousarray(np.eye(128, dtype=F32).astype(BF16))

    x_hi = x_flat.astype(BF16)
    x_lo = (x_flat - x_hi.astype(F32)).astype(BF16)
    in_maps = []
    for c in range(NCORES):
        sl = slice(c * NT, (c + 1) * NT)
        in_maps.append(
            {"xTh": np.ascontiguousarray(x_hi[sl].T),
             "xTl": np.ascontiguousarray(x_lo[sl].T),
             "wT": wT, "HmS": HmS, "laE": laE,
             "lbTe": lbTe, "ident": ident}
        )
    return in_maps


def kernel(x, S_in, H_block, w_quantized, lora_a, lora_b, bias):
    nc = _get_nc()
    in_maps = make_in_maps(x, S_in, H_block, w_quantized, lora_a, lora_b, bias)
    res = run_bass_kernel_spmd(nc, in_maps, core_ids=list(range(NCORES)))
    out = np.concatenate([res.results[c]["y"] for c in range(NCORES)], axis=0)
    return out.reshape(B, S, D_OUT).astype(F32)


# revision 15
# speedup vs baseline: 1.1049x; 1.1049x over previous
"""CascadeHadamardLinear Trainium2 kernel (8-core data-parallel over tokens).

Math per token row x[4096]:
  x_rot = (x * S_in) @ blockdiag(H_128)
  x_q   = NVFP4 fake-quant of x_rot (16-elem blocks, e2m1 snap, RNE)
  out   = x_q @ W^T + (x_rot @ la^T) @ lb^T + bias

Host side: shard 8192 tokens over 8 cores (1024 each). x ships as a
bf16 hi/lo pair (x = hi + lo exactly to ~2^-17), the Hadamard as the
exact-in-bf16 sign matrix (H*sqrt(128))*S per block, so rotation runs
as two accumulating bf16 matmuls per tile with 1/sqrt(128) folded into
the quant scale constant. la_eff ( = per-block diag(S) H la^T ) is host
precomputed; lbT carries a 33rd row = bias matched by a ones-row in
t1T so bias needs no extra matmul.

Device, per core, tokens in 4 quarters of 256 (P1), output in 8 column
groups of 512 (P2, W loaded once, on the scalar HWDGE queue so it
doesn't sit behind P1's x loads on sync):
  P1(q) per block-group jg: rotation MMs (hi+lo) -> PSUM bank
    [128t,512] fp32; ACT evac to SBUF; quant on SBUF: absmax-16 + recip
    + x6 (DVE), z = b*rs6 (GPSIMD), two fused custom DVE snap ops
    (magic-add RNE + selects on z^2), xq = f*sc -> bf16 (GPSIMD);
    LoRA1 via 4-way column-tiled concurrent MMs (tile_position),
    group-summed by a small select matmul per quarter; PE-transpose
    xq -> xqT.
  P2 chain (og,th): PSUM accum of 32 xqT MMs + [t1T;1]^T @ [lbT;bias],
    ACT evac bf16, DMA out. Chains for og0/og1 are injected between P1
    jg steps (3 per quarter) so the in-order PE queue stays dense while
    DVE/GPSIMD run the quant chain; the rest follow P1.
"""

import os
import sys

for _p in ("/opt/trn_rl_repo",):
    if os.path.isdir(_p) and _p not in sys.path:
        sys.path.insert(0, _p)

import numpy as np

import concourse.bass as bass
import concourse.mybir as mybir
import concourse.tile as tile
from concourse import bacc
from concourse.bass_utils import run_bass_kernel_spmd

F32 = np.float32

# ---------------- problem constants (hardcoded per contract) ----------------
B, S, D_IN, D_OUT, RANK, HBS = 4, 2048, 4096, 4096, 32, 128
NTOK = B * S                  # 8192
NCORES = 8
NT = NTOK // NCORES           # 1024 tokens per core
NJ = D_IN // HBS              # 32 hadamard blocks
QB = 16                       # quant block size
NQ = 4                        # token quarters per core
QT = NT // NQ                 # 256 tokens per quarter
OG_N = D_OUT // 512           # 8 output column groups

# 1/sqrt(128) exactly as the reference's fp32 H entries have it
H_SCALE = float(np.float32(np.float64(1.0) / np.sqrt(np.float64(128.0))))

# quant snap constants (1.5*2^k magic so ulp is uniform on both sides of c)
C_INT = 12582912.0            # 1.5*2^23, ulp 1.0
C_HALF = 6291456.0            # 1.5*2^22, ulp 0.5
TH23 = 20.25                  # 4.5^2
THF = 5.0625                  # 2.25^2

# ---------------- custom DVE ops (e2m1 level snap) ----------------
def _register_snap_ops():
    from concourse.dve_spec import (
        Spec, Src0, Src1, C0, C1, C2, lower as dve_lower, sq, select, _has_src1,
    )
    from concourse.dve_ops import (
        DveOp, OPS, CUSTOM_DVE_SPECS, _SUB_OPCODE_FOR_NAME, _CUSTOM_DVE_ROW_BASE,
    )
    from concourse.dve_uop import DveOpSpec
    from concourse.dve_table_gen import dve_ver_for

    def _ref_midlow(in0, in1, c0, c1, c2):
        z = in0.astype(F32)
        c1 = F32(c1)
        c2 = F32(c2)
        th = (z + c1) - c1
        ti = (z + c2) - c2
        return np.where(z * z < F32(c0), th, ti).astype(F32)

    def _ref_sel23(in0, in1, c0, c1, c2):
        z = in0.astype(F32)
        c3 = F32(2.0) * F32(c1)
        t3 = (z + c3) - c3
        return np.where(z * z < F32(c0), in1.astype(F32), t3).astype(F32)

    def _mk(name, body, ref):
        if name in _SUB_OPCODE_FOR_NAME:
            return next(op for op in OPS if op.name == name)
        spec = Spec(body=body, reference=ref)
        row = _CUSTOM_DVE_ROW_BASE + len(OPS)
        assert row < 0x20
        ver = dve_ver_for("TRN2")
        uops = dve_lower(spec, ver=ver)
        sha = DveOpSpec(
            name=name, opcode=row, uops=uops, rd1_en=_has_src1(spec)
        ).sha(ver)
        op = DveOp(name, spec, subdim=False, uops_sha={ver: sha})
        OPS.append(op)
        CUSTOM_DVE_SPECS[name] = spec
        _SUB_OPCODE_FOR_NAME[name] = row
        return op

    z = Src0
    midlow = _mk(
        "SNAP_MIDLOW_ANT",
        select(sq(z) < C0, (z + C1) - C1, (z + C2) - C2),
        _ref_midlow,
    )
    c3 = C1 + C1
    sel23 = _mk(
        "SNAP_SEL23_ANT",
        select(sq(z) < C0, Src1, (z + c3) - c3),
        _ref_sel23,
    )
    return midlow, sel23


SNAP_MIDLOW, SNAP_SEL23 = _register_snap_ops()


# ---------------- device kernel ----------------
def _build_nc():
    nc = bacc.Bacc(
        "TRN2", target_bir_lowering=False, debug=False, num_devices=NCORES
    )
    dt = mybir.dt
    # x pre-arranged on host: [jg, c, q, j, hi/lo, t] so each (jg, q) slice is
    # one fully-contiguous 2KB-per-partition DMA
    xThl = nc.dram_tensor(
        "xThl", [NJ // 4, HBS, NQ, 4, 2, QT], dt.bfloat16, kind="ExternalInput"
    )
    wT = nc.dram_tensor("wT", [D_IN, D_OUT], dt.bfloat16, kind="ExternalInput")
    HmS = nc.dram_tensor("HmS", [HBS, NJ, HBS], dt.bfloat16, kind="ExternalInput")
    laE = nc.dram_tensor("laE", [HBS, NJ, RANK], dt.bfloat16, kind="ExternalInput")
    lbTe = nc.dram_tensor("lbTe", [RANK + 1, D_OUT], dt.bfloat16, kind="ExternalInput")
    ident = nc.dram_tensor("ident", [128, 128], dt.bfloat16, kind="ExternalInput")
    sel4 = nc.dram_tensor("sel4", [128, RANK], dt.bfloat16, kind="ExternalInput")
    y = nc.dram_tensor("y", [NT, D_OUT], dt.bfloat16, kind="ExternalOutput")

    with tile.TileContext(nc) as tc:
        _emit(nc, tc, xThl, wT, HmS, laE, lbTe, ident, sel4, y)
    nc.compile()
    return nc


def _emit(nc, tc, xThl, wT, HmS, laE, lbTe, ident, sel4, y):
    from contextlib import ExitStack

    dt = mybir.dt
    Alu = mybir.AluOpType

    with ExitStack() as ctx:
        consts = ctx.enter_context(tc.tile_pool(name="consts", bufs=1))
        xqT_pool = ctx.enter_context(tc.tile_pool(name="xqT", bufs=1))

        H_sb = consts.tile([HBS, NJ, HBS], dt.bfloat16)
        nc.sync.dma_start(out=H_sb[:], in_=HmS[:])
        la_sb = consts.tile([HBS, NJ, RANK], dt.bfloat16)
        nc.sync.dma_start(out=la_sb[:], in_=laE[:])
        lb_sb = consts.tile([RANK + 1, D_OUT], dt.bfloat16)
        nc.sync.dma_start(out=lb_sb[:], in_=lbTe[:])
        id_sb = consts.tile([128, 128], dt.bfloat16)
        nc.sync.dma_start(out=id_sb[:], in_=ident[:])
        sel4_sb = consts.tile([128, RANK], dt.bfloat16)
        nc.sync.dma_start(out=sel4_sb[:], in_=sel4[:])
        t1T = consts.tile([RANK + 1, NT], dt.bfloat16)
        nc.vector.memset(t1T[RANK : RANK + 1, :], 1.0)

        # xqT[c, j, t] : feature-major quantized activations (bf16)
        xqT = xqT_pool.tile([128, NJ, NT], dt.bfloat16)

        xt_pool = ctx.enter_context(tc.tile_pool(name="xt", bufs=3))
        bsb_pool = ctx.enter_context(tc.tile_pool(name="bsb", bufs=3))
        qsm = ctx.enter_context(tc.tile_pool(name="qsm", bufs=8))
        qtmp = ctx.enter_context(tc.tile_pool(name="qtmp", bufs=6))
        xq_pool = ctx.enter_context(tc.tile_pool(name="xq", bufs=4))
        t1b_pool = ctx.enter_context(tc.tile_pool(name="t1b", bufs=2))
        wbf_pool = ctx.enter_context(tc.tile_pool(name="wbf", bufs=2))
        out_pool = ctx.enter_context(tc.tile_pool(name="out", bufs=4))
        rot_ps = ctx.enter_context(tc.tile_pool(name="rotps", bufs=2, space="PSUM"))
        pt_ps = ctx.enter_context(tc.tile_pool(name="ptps", bufs=2, space="PSUM"))
        t1_ps = ctx.enter_context(tc.tile_pool(name="t1ps", bufs=1, space="PSUM"))
        out_ps = ctx.enter_context(tc.tile_pool(name="outps", bufs=2, space="PSUM"))

        def load_wbf(og):
            osl = slice(og * 512, (og + 1) * 512)
            wbf = wbf_pool.tile([128, NJ, 512], dt.bfloat16, name=f"wbf{og}", tag="wbf")
            # scalar (ACT) hwdge queue: decoupled from the xt stream on sync
            nc.scalar.dma_start(
                out=wbf[:], in_=wT[:, osl].rearrange("(j c) o -> c j o", c=HBS)
            )
            return wbf

        def emit_chain(og, th, wbf):
            osl = slice(og * 512, (og + 1) * 512)
            tsl = slice(th * 128, (th + 1) * 128)
            po = out_ps.tile([128, 512], dt.float32, name=f"po{og}_{th}", tag="po")
            for j in range(NJ):
                nc.tensor.matmul(
                    po[:], lhsT=xqT[:, j, tsl], rhs=wbf[:, j, :],
                    start=(j == 0), stop=False,
                )
            nc.tensor.matmul(
                po[:], lhsT=t1T[:, tsl], rhs=lb_sb[:, osl],
                start=False, stop=True,
            )
            ot = out_pool.tile([128, 512], dt.bfloat16, name=f"ot{og}_{th}", tag="ot")
            nc.scalar.copy(out=ot[:], in_=po[:])
            nc.scalar.dma_start(out=y[tsl, osl], in_=ot[:])

        def emit_p1(q, inject):
            qsl = slice(q * QT, (q + 1) * QT)
            t1p = t1_ps.tile([RANK, QT], dt.float32, name=f"t1p{q}", tag="t1p")
            for jg in range(NJ // 4):
                xt = xt_pool.tile([128, 4, 2, QT], dt.bfloat16,
                                  name=f"xt{q}_{jg}", tag="xt")
                nc.sync.dma_start(out=xt[:], in_=xThl[jg, :, q, :, :, :])
                # LoRA1 accumulation over all 32 j blocks
                for dj in range(4):
                    j = 4 * jg + dj
                    nc.tensor.matmul(
                        t1p[:], lhsT=la_sb[:, j, :], rhs=xt[:, dj, 0, :],
                        start=(j == 0), stop=(j == NJ - 1),
                    )
                xq_tiles = []
                for ts in range(QT // 128):
                    bank = rot_ps.tile([128, 512], dt.float32,
                                       name=f"bank{q}{jg}{ts}", tag="bank")
                    for dj in range(4):
                        j = 4 * jg + dj
                        for hl in range(2):
                            nc.tensor.matmul(
                                bank[:, dj * HBS : (dj + 1) * HBS],
                                lhsT=xt[:, dj, hl, ts * 128 : (ts + 1) * 128],
                                rhs=H_sb[:, j, :],
                                start=(dj == 0 and hl == 0),
                                stop=(dj == 3 and hl == 1),
                            )
                    bsb = bsb_pool.tile([128, 512], dt.float32,
                                        name=f"bsb{q}{jg}{ts}", tag="bsb")
                    nc.scalar.copy(out=bsb[:], in_=bank[:])
                    nb = 512 // QB
                    amax = qsm.tile([128, nb], dt.float32, name=f"amax{q}{jg}{ts}", tag="amax")
                    nc.vector.tensor_reduce(
                        out=amax[:], in_=bsb[:].rearrange("p (b s) -> p b s", s=QB),
                        axis=mybir.AxisListType.X, op=Alu.max,
                        apply_absolute_value=True,
                    )
                    ra = qsm.tile([128, nb], dt.float32, name=f"ra{q}{jg}{ts}", tag="ra")
                    nc.vector.reciprocal(out=ra[:], in_=amax[:])
                    rs6 = qsm.tile([128, nb], dt.float32, name=f"rs6{q}{jg}{ts}", tag="rs6")
                    nc.vector.tensor_scalar(
                        out=rs6[:], in0=ra[:], scalar1=6.0, scalar2=None,
                        op0=Alu.mult,
                    )
                    sc = qsm.tile([128, nb], dt.float32, name=f"sc{q}{jg}{ts}", tag="sc")
                    nc.scalar.mul(out=sc[:], in_=amax[:], mul=H_SCALE / 6.0)
                    z = qtmp.tile([128, 512], dt.float32, name=f"z{q}{jg}{ts}", tag="qt")
                    nc.gpsimd.tensor_tensor(
                        out=z[:].rearrange("p (b s) -> p b s", s=QB),
                        in0=bsb[:].rearrange("p (b s) -> p b s", s=QB),
                        in1=rs6[:].unsqueeze(2).broadcast_to([128, nb, QB]),
                        op=Alu.mult,
                    )
                    r = qtmp.tile([128, 512], dt.float32, name=f"r{q}{jg}{ts}", tag="qt")
                    nc.vector._custom_dve(
                        SNAP_MIDLOW, out=r[:], in0=z[:], s0=THF, s1=C_HALF,
                        imm2=C_INT,
                    )
                    f = qtmp.tile([128, 512], dt.float32, name=f"f{q}{jg}{ts}", tag="qt")
                    nc.vector._custom_dve(
                        SNAP_SEL23, out=f[:], in0=z[:], in1=r[:], s0=TH23, s1=C_INT,
                    )
                    xq_t = xq_pool.tile([128, 512], dt.bfloat16, name=f"xq{q}{jg}{ts}", tag="xq")
                    nc.gpsimd.tensor_tensor(
                        out=xq_t[:].rearrange("p (b s) -> p b s", s=QB),
                        in0=f[:].rearrange("p (b s) -> p b s", s=QB),
                        in1=sc[:].unsqueeze(2).broadcast_to([128, nb, QB]),
                        op=Alu.mult,
                    )
                    xq_tiles.append(xq_t)
                # inject a ready P2 chain between this jg's rotation and its
                # transposes: the transposes wait on the quant chain anyway,
                # so the in-order PE queue stays dense
                if jg in (1, 3, 5):
                    inject()
                pt = pt_ps.tile([128, 4, QT], dt.bfloat16, name=f"pt{q}_{jg}", tag="pt")
                nts = QT // 128
                for dj in range(4):
                    for ts in range(nts):
                        nc.tensor.matmul(
                            pt[:, dj, ts * 128 : (ts + 1) * 128],
                            lhsT=xq_tiles[ts][:, dj * HBS : (dj + 1) * HBS],
                            rhs=id_sb[:], is_transpose=True,
                            start=(dj == 0 and ts == 0),
                            stop=(dj == 3 and ts == nts - 1),
                        )
                nc.scalar.copy(out=xqT[:, 4 * jg : 4 * jg + 4, qsl], in_=pt[:])
            nc.scalar.copy(out=t1T[0:RANK, qsl], in_=t1p[:])

        # P2 chains for og0/og1 injected into P1; W preloaded up front
        wbf_pre = [load_wbf(0), load_wbf(1)]
        pending = [(og, th) for th in range(NT // 128) for og in (0, 1)]
        pos = 0

        def inject():
            nonlocal pos
            if pos < len(pending):
                og, th = pending[pos]
                pos += 1
                emit_chain(og, th, wbf_pre[og])

        def no_inject():
            pass

        for q in range(NQ):
            # chains for tokens of quarter q-1 and earlier are ready
            emit_p1(q, inject if q > 0 else no_inject)
        # leftovers of og0/og1, then the remaining column groups
        while pos < len(pending):
            og, th = pending[pos]
            pos += 1
            emit_chain(og, th, wbf_pre[og])
        for og in range(2, OG_N):
            wbf = load_wbf(og)
            for th in range(NT // 128):
                emit_chain(og, th, wbf)


_NC_CACHE = None


def _get_nc():
    global _NC_CACHE
    if _NC_CACHE is None:
        _NC_CACHE = _build_nc()
    return _NC_CACHE


# ---------------- host wrapper ----------------
def make_in_maps(x, S_in, H_block, w_quantized, lora_a, lora_b, bias):
    import ml_dtypes
    BF16 = ml_dtypes.bfloat16

    x = np.asarray(x, dtype=F32)
    S_in = np.asarray(S_in, dtype=F32)
    H_block = np.asarray(H_block, dtype=F32)
    w_quantized = np.asarray(w_quantized, dtype=F32)
    lora_a = np.asarray(lora_a, dtype=F32)
    lora_b = np.asarray(lora_b, dtype=F32)
    bias = np.asarray(bias, dtype=F32)

    x_flat = x.reshape(NTOK, D_IN)
    wT = np.ascontiguousarray(w_quantized.T.astype(BF16))   # [D_IN, D_OUT]

    # sign matrix: H_block = Hpm * (1/sqrt(128)); Hpm entries are +-1 (bf16 exact)
    Hpm = np.where(H_block > 0, np.float32(1.0), np.float32(-1.0))
    Sc = S_in.reshape(NJ, HBS)                              # [j, r]
    # HmS[r, j, c] = Hpm[r, c] * S[j*128+r]
    HmS = np.ascontiguousarray(
        (Hpm[None, :, :] * Sc[:, :, None]).transpose(1, 0, 2).astype(BF16)
    )
    # la_eff[c, j, r] = S[j*128+c] * sum_k H_block[k, c] * lora_a[r, j*128+k]
    la_blk = lora_a.reshape(RANK, NJ, HBS)                  # [r, j, k]
    la_rot = np.einsum("kc,rjk->cjr", H_block, la_blk)      # [c, j, r]
    laE = np.ascontiguousarray((la_rot * Sc.T[:, :, None]).astype(BF16))
    lbTe = np.ascontiguousarray(
        np.concatenate([lora_b.T, bias.reshape(1, D_OUT)], axis=0).astype(BF16)
    )
    ident = np.ascontiguousarray(np.eye(128, dtype=F32).astype(BF16))
    sel4 = np.ascontiguousarray(
        np.tile(np.eye(RANK, dtype=F32), (4, 1)).astype(BF16)
    )

    x_hi = x_flat.astype(BF16)
    x_lo = (x_flat - x_hi.astype(F32)).astype(BF16)
    in_maps = []
    for c in range(NCORES):
        sl = slice(c * NT, (c + 1) * NT)
        # [h, q, t, jg, j, c] -> [jg, c, q, j, h, t]
        xhl = np.stack([x_hi[sl], x_lo[sl]], axis=0).reshape(
            2, NQ, QT, NJ // 4, 4, HBS
        ).transpose(3, 5, 1, 4, 0, 2)
        in_maps.append(
            {"xThl": np.ascontiguousarray(xhl),
             "wT": wT, "HmS": HmS, "laE": laE,
             "lbTe": lbTe, "ident": ident, "sel4": sel4}
        )
    return in_maps


def kernel(x, S_in, H_block, w_quantized, lora_a, lora_b, bias):
    nc = _get_nc()
    in_maps = make_in_maps(x, S_in, H_block, w_quantized, lora_a, lora_b, bias)
    res = run_bass_kernel_spmd(nc, in_maps, core_ids=list(range(NCORES)))
    out = np.concatenate([res.results[c]["y"] for c in range(NCORES)], axis=0)
    return out.reshape(B, S, D_OUT).astype(F32)


# revision 18
# speedup vs baseline: 1.1185x; 1.0123x over previous
"""CascadeHadamardLinear Trainium2 kernel (8-core data-parallel over tokens).

Math per token row x[4096]:
  x_rot = (x * S_in) @ blockdiag(H_128)
  x_q   = NVFP4 fake-quant of x_rot (16-elem blocks, e2m1 snap, RNE)
  out   = x_q @ W^T + (x_rot @ la^T) @ lb^T + bias

Host side: shard 8192 tokens over 8 cores (1024 each). x ships as a
bf16 hi/lo pair (x = hi + lo exactly to ~2^-17), the Hadamard as the
exact-in-bf16 sign matrix (H*sqrt(128))*S per block, so rotation runs
as two accumulating bf16 matmuls per tile with 1/sqrt(128) folded into
the quant scale constant. la_eff ( = per-block diag(S) H la^T ) is host
precomputed; lbT carries a 33rd row = bias matched by a ones-row in
t1T so bias needs no extra matmul.

Device, per core, tokens in 4 quarters of 256 (P1), output in 8 column
groups of 512 (P2, W loaded once, on the scalar HWDGE queue so it
doesn't sit behind P1's x loads on sync):
  P1(q) per block-group jg: rotation MMs (hi+lo) -> PSUM bank
    [128t,512] fp32; ACT evac to SBUF; quant on SBUF: absmax-16 + recip
    + x6 (DVE), z = b*rs6 (GPSIMD), two fused custom DVE snap ops
    (magic-add RNE + selects on z^2), xq = f*sc -> bf16 (GPSIMD);
    LoRA1 via 4-way column-tiled concurrent MMs (tile_position),
    group-summed by a small select matmul per quarter; PE-transpose
    xq -> xqT.
  P2 chain (og,th): PSUM accum of 32 xqT MMs + [t1T;1]^T @ [lbT;bias],
    ACT evac bf16, DMA out. Chains for og0/og1 are injected between P1
    jg steps (3 per quarter) so the in-order PE queue stays dense while
    DVE/GPSIMD run the quant chain; the rest follow P1.
"""

import os
import sys

for _p in ("/opt/trn_rl_repo",):
    if os.path.isdir(_p) and _p not in sys.path:
        sys.path.insert(0, _p)

import numpy as np

import concourse.bass as bass
import concourse.mybir as mybir
import concourse.tile as tile
from concourse import bacc
from concourse.bass_utils import run_bass_kernel_spmd

F32 = np.float32

# ---------------- problem constants (hardcoded per contract) ----------------
B, S, D_IN, D_OUT, RANK, HBS = 4, 2048, 4096, 4096, 32, 128
NTOK = B * S                  # 8192
NCORES = 8
NT = NTOK // NCORES           # 1024 tokens per core
NJ = D_IN // HBS              # 32 hadamard blocks
QB = 16                       # quant block size
NQ = 4                        # token quarters per core
QT = NT // NQ                 # 256 tokens per quarter
OG_N = D_OUT // 512           # 8 output column groups

# 1/sqrt(128) exactly as the reference's fp32 H entries have it
H_SCALE = float(np.float32(np.float64(1.0) / np.sqrt(np.float64(128.0))))

# quant snap constants (1.5*2^k magic so ulp is uniform on both sides of c)
C_INT = 12582912.0            # 1.5*2^23, ulp 1.0
C_HALF = 6291456.0            # 1.5*2^22, ulp 0.5
TH23 = 20.25                  # 4.5^2
THF = 5.0625                  # 2.25^2

# ---------------- custom DVE ops (e2m1 level snap) ----------------
def _register_snap_ops():
    from concourse.dve_spec import (
        Spec, Src0, Src1, C0, C1, C2, lower as dve_lower, sq, select, _has_src1,
    )
    from concourse.dve_ops import (
        DveOp, OPS, CUSTOM_DVE_SPECS, _SUB_OPCODE_FOR_NAME, _CUSTOM_DVE_ROW_BASE,
    )
    from concourse.dve_uop import DveOpSpec
    from concourse.dve_table_gen import dve_ver_for

    def _ref_midlow(in0, in1, c0, c1, c2):
        z = in0.astype(F32)
        c1 = F32(c1)
        c2 = F32(c2)
        th = (z + c1) - c1
        ti = (z + c2) - c2
        return np.where(z * z < F32(c0), th, ti).astype(F32)

    def _ref_sel23(in0, in1, c0, c1, c2):
        z = in0.astype(F32)
        c3 = F32(2.0) * F32(c1)
        t3 = (z + c3) - c3
        return np.where(z * z < F32(c0), in1.astype(F32), t3).astype(F32)

    def _mk(name, body, ref):
        if name in _SUB_OPCODE_FOR_NAME:
            return next(op for op in OPS if op.name == name)
        spec = Spec(body=body, reference=ref)
        row = _CUSTOM_DVE_ROW_BASE + len(OPS)
        assert row < 0x20
        ver = dve_ver_for("TRN2")
        uops = dve_lower(spec, ver=ver)
        sha = DveOpSpec(
            name=name, opcode=row, uops=uops, rd1_en=_has_src1(spec)
        ).sha(ver)
        op = DveOp(name, spec, subdim=False, uops_sha={ver: sha})
        OPS.append(op)
        CUSTOM_DVE_SPECS[name] = spec
        _SUB_OPCODE_FOR_NAME[name] = row
        return op

    z = Src0
    midlow = _mk(
        "SNAP_MIDLOW_ANT",
        select(sq(z) < C0, (z + C1) - C1, (z + C2) - C2),
        _ref_midlow,
    )
    c3 = C1 + C1
    sel23 = _mk(
        "SNAP_SEL23_ANT",
        select(sq(z) < C0, Src1, (z + c3) - c3),
        _ref_sel23,
    )
    return midlow, sel23


SNAP_MIDLOW, SNAP_SEL23 = _register_snap_ops()


# ---------------- device kernel ----------------
def _build_nc():
    nc = bacc.Bacc(
        "TRN2", target_bir_lowering=False, debug=False, num_devices=NCORES
    )
    dt = mybir.dt
    # x pre-arranged on host: [jg, c, q, j, hi/lo, t] so each (jg, q) slice is
    # one fully-contiguous 2KB-per-partition DMA
    xThl = nc.dram_tensor(
        "xThl", [NJ // 4, HBS, NQ, 4, 2, QT], dt.bfloat16, kind="ExternalInput"
    )
    wT = nc.dram_tensor("wT", [D_IN, D_OUT], dt.bfloat16, kind="ExternalInput")
    HmS = nc.dram_tensor("HmS", [HBS, NJ, HBS], dt.bfloat16, kind="ExternalInput")
    laE = nc.dram_tensor("laE", [HBS, NJ, RANK], dt.bfloat16, kind="ExternalInput")
    lbTe = nc.dram_tensor("lbTe", [RANK + 1, D_OUT], dt.bfloat16, kind="ExternalInput")
    ident = nc.dram_tensor("ident", [128, 128], dt.bfloat16, kind="ExternalInput")
    sel4 = nc.dram_tensor("sel4", [128, RANK], dt.bfloat16, kind="ExternalInput")
    y = nc.dram_tensor("y", [NT, D_OUT], dt.bfloat16, kind="ExternalOutput")

    with tile.TileContext(nc) as tc:
        _emit(nc, tc, xThl, wT, HmS, laE, lbTe, ident, sel4, y)
    nc.compile()
    return nc


def _emit(nc, tc, xThl, wT, HmS, laE, lbTe, ident, sel4, y):
    from contextlib import ExitStack

    dt = mybir.dt
    Alu = mybir.AluOpType

    with ExitStack() as ctx:
        consts = ctx.enter_context(tc.tile_pool(name="consts", bufs=1))
        xqT_pool = ctx.enter_context(tc.tile_pool(name="xqT", bufs=1))

        H_sb = consts.tile([HBS, NJ, HBS], dt.bfloat16)
        nc.sync.dma_start(out=H_sb[:], in_=HmS[:])
        la_sb = consts.tile([HBS, NJ, RANK], dt.bfloat16)
        nc.sync.dma_start(out=la_sb[:], in_=laE[:])
        lb_sb = consts.tile([RANK + 1, D_OUT], dt.bfloat16)
        nc.sync.dma_start(out=lb_sb[:], in_=lbTe[:])
        id_sb = consts.tile([128, 128], dt.bfloat16)
        nc.sync.dma_start(out=id_sb[:], in_=ident[:])
        sel4_sb = consts.tile([128, RANK], dt.bfloat16)
        nc.sync.dma_start(out=sel4_sb[:], in_=sel4[:])
        t1T = consts.tile([RANK + 1, NT], dt.bfloat16)
        nc.vector.memset(t1T[RANK : RANK + 1, :], 1.0)

        # xqT[c, j, t] : feature-major quantized activations (bf16)
        xqT = xqT_pool.tile([128, NJ, NT], dt.bfloat16)

        xt_pool = ctx.enter_context(tc.tile_pool(name="xt", bufs=3))
        bsb_pool = ctx.enter_context(tc.tile_pool(name="bsb", bufs=4))
        qsm = ctx.enter_context(tc.tile_pool(name="qsm", bufs=12))
        qtmp = ctx.enter_context(tc.tile_pool(name="qtmp", bufs=8))
        xq_pool = ctx.enter_context(tc.tile_pool(name="xq", bufs=6))
        wbf_pool = ctx.enter_context(tc.tile_pool(name="wbf", bufs=2))
        out_pool = ctx.enter_context(tc.tile_pool(name="out", bufs=4))
        rot_ps = ctx.enter_context(tc.tile_pool(name="rotps", bufs=3, space="PSUM"))
        pt_ps = ctx.enter_context(tc.tile_pool(name="ptps", bufs=2, space="PSUM"))
        t1_ps = ctx.enter_context(tc.tile_pool(name="t1ps", bufs=1, space="PSUM"))
        out_ps = ctx.enter_context(tc.tile_pool(name="outps", bufs=2, space="PSUM"))

        def load_wbf(og):
            osl = slice(og * 512, (og + 1) * 512)
            wbf = wbf_pool.tile([128, NJ, 512], dt.bfloat16, name=f"wbf{og}", tag="wbf")
            # scalar (ACT) hwdge queue: decoupled from the xt stream on sync
            nc.scalar.dma_start(
                out=wbf[:], in_=wT[:, osl].rearrange("(j c) o -> c j o", c=HBS)
            )
            return wbf

        def emit_chain(og, th, wbf):
            osl = slice(og * 512, (og + 1) * 512)
            tsl = slice(th * 128, (th + 1) * 128)
            po = out_ps.tile([128, 512], dt.float32, name=f"po{og}_{th}", tag="po")
            # lora2+bias first: its t1T weight-load hides under the previous
            # chain's tail instead of stalling this chain's end
            nc.tensor.matmul(
                po[:], lhsT=t1T[:, tsl], rhs=lb_sb[:, osl],
                start=True, stop=False,
            )
            for j in range(NJ):
                nc.tensor.matmul(
                    po[:], lhsT=xqT[:, j, tsl], rhs=wbf[:, j, :],
                    start=False, stop=(j == NJ - 1),
                )
            ot = out_pool.tile([128, 512], dt.bfloat16, name=f"ot{og}_{th}", tag="ot")
            nc.scalar.copy(out=ot[:], in_=po[:])
            nc.scalar.dma_start(out=y[tsl, osl], in_=ot[:])

        def emit_p1(q, inject):
            qsl = slice(q * QT, (q + 1) * QT)
            t1p = t1_ps.tile([RANK, QT], dt.float32, name=f"t1p{q}", tag="t1p")
            for jg in range(NJ // 4):
                xt = xt_pool.tile([128, 4, 2, QT], dt.bfloat16,
                                  name=f"xt{q}_{jg}", tag="xt")
                nc.sync.dma_start(out=xt[:], in_=xThl[jg, :, q, :, :, :])
                # LoRA1 accumulation over all 32 j blocks
                for dj in range(4):
                    j = 4 * jg + dj
                    nc.tensor.matmul(
                        t1p[:], lhsT=la_sb[:, j, :], rhs=xt[:, dj, 0, :],
                        start=(j == 0), stop=(j == NJ - 1),
                    )
                xq_tiles = []
                for ts in range(QT // 128):
                    bank = rot_ps.tile([128, 512], dt.float32,
                                       name=f"bank{q}{jg}{ts}", tag="bank")
                    for dj in range(4):
                        j = 4 * jg + dj
                        for hl in range(2):
                            nc.tensor.matmul(
                                bank[:, dj * HBS : (dj + 1) * HBS],
                                lhsT=xt[:, dj, hl, ts * 128 : (ts + 1) * 128],
                                rhs=H_sb[:, j, :],
                                start=(dj == 0 and hl == 0),
                                stop=(dj == 3 and hl == 1),
                            )
                    bsb = bsb_pool.tile([128, 512], dt.float32,
                                        name=f"bsb{q}{jg}{ts}", tag="bsb")
                    nc.scalar.copy(out=bsb[:], in_=bank[:])
                    nb = 512 // QB
                    amax = qsm.tile([128, nb], dt.float32, name=f"amax{q}{jg}{ts}", tag="amax")
                    nc.vector.tensor_reduce(
                        out=amax[:], in_=bsb[:].rearrange("p (b s) -> p b s", s=QB),
                        axis=mybir.AxisListType.X, op=Alu.max,
                        apply_absolute_value=True,
                    )
                    ra = qsm.tile([128, nb], dt.float32, name=f"ra{q}{jg}{ts}", tag="ra")
                    nc.vector.reciprocal(out=ra[:], in_=amax[:])
                    rs6 = qsm.tile([128, nb], dt.float32, name=f"rs6{q}{jg}{ts}", tag="rs6")
                    nc.vector.tensor_scalar(
                        out=rs6[:], in0=ra[:], scalar1=6.0, scalar2=None,
                        op0=Alu.mult,
                    )
                    sc = qsm.tile([128, nb], dt.float32, name=f"sc{q}{jg}{ts}", tag="sc")
                    nc.scalar.mul(out=sc[:], in_=amax[:], mul=H_SCALE / 6.0)
                    z = qtmp.tile([128, 512], dt.float32, name=f"z{q}{jg}{ts}", tag="qt")
                    nc.gpsimd.tensor_tensor(
                        out=z[:].rearrange("p (b s) -> p b s", s=QB),
                        in0=bsb[:].rearrange("p (b s) -> p b s", s=QB),
                        in1=rs6[:].unsqueeze(2).broadcast_to([128, nb, QB]),
                        op=Alu.mult,
                    )
                    r = qtmp.tile([128, 512], dt.float32, name=f"r{q}{jg}{ts}", tag="qt")
                    nc.vector._custom_dve(
                        SNAP_MIDLOW, out=r[:], in0=z[:], s0=THF, s1=C_HALF,
                        imm2=C_INT,
                    )
                    f = qtmp.tile([128, 512], dt.float32, name=f"f{q}{jg}{ts}", tag="qt")
                    nc.vector._custom_dve(
                        SNAP_SEL23, out=f[:], in0=z[:], in1=r[:], s0=TH23, s1=C_INT,
                    )
                    xq_t = xq_pool.tile([128, 512], dt.bfloat16, name=f"xq{q}{jg}{ts}", tag="xq")
                    nc.gpsimd.tensor_tensor(
                        out=xq_t[:].rearrange("p (b s) -> p b s", s=QB),
                        in0=f[:].rearrange("p (b s) -> p b s", s=QB),
                        in1=sc[:].unsqueeze(2).broadcast_to([128, nb, QB]),
                        op=Alu.mult,
                    )
                    xq_tiles.append(xq_t)
                # inject a ready P2 chain between this jg's rotation and its
                # transposes: the transposes wait on the quant chain anyway,
                # so the in-order PE queue stays dense
                if jg in (1, 3, 5):
                    inject()
                pt = pt_ps.tile([128, 4, QT], dt.bfloat16, name=f"pt{q}_{jg}", tag="pt")
                nts = QT // 128
                for dj in range(4):
                    for ts in range(nts):
                        nc.tensor.matmul(
                            pt[:, dj, ts * 128 : (ts + 1) * 128],
                            lhsT=xq_tiles[ts][:, dj * HBS : (dj + 1) * HBS],
                            rhs=id_sb[:], is_transpose=True,
                            start=(dj == 0 and ts == 0),
                            stop=(dj == 3 and ts == nts - 1),
                        )
                nc.scalar.copy(out=xqT[:, 4 * jg : 4 * jg + 4, qsl], in_=pt[:])
            nc.scalar.copy(out=t1T[0:RANK, qsl], in_=t1p[:])

        # P2 chains for og0/og1 injected into P1; W preloaded up front
        wbf_pre = [load_wbf(0), load_wbf(1)]
        pending = [(og, th) for th in range(NT // 128) for og in (0, 1)]
        pos = 0

        def inject():
            nonlocal pos
            if pos < len(pending):
                og, th = pending[pos]
                pos += 1
                emit_chain(og, th, wbf_pre[og])

        def no_inject():
            pass

        for q in range(NQ):
            # chains for tokens of quarter q-1 and earlier are ready
            emit_p1(q, inject if q > 0 else no_inject)
        # leftovers: og0 first so its wbf slot frees for og2's prefetch,
        # og1's leftovers then hide og2's transfer
        left = [pt for pt in pending[pos:] if pt[0] == 0] + \
               [pt for pt in pending[pos:] if pt[0] == 1]
        mid = len([pt for pt in left if pt[0] == 0])
        for i, (og, th) in enumerate(left):
            if i == mid:
                wbf_next = load_wbf(2)
            emit_chain(og, th, wbf_pre[og])
        if mid == len(left):
            wbf_next = load_wbf(2)
        for og in range(2, OG_N):
            wbf = wbf_next
            wbf_next = load_wbf(og + 1) if og + 1 < OG_N else None
            for th in range(NT // 128):
                emit_chain(og, th, wbf)


_NC_CACHE = None


def _get_nc():
    global _NC_CACHE
    if _NC_CACHE is None:
        _NC_CACHE = _build_nc()
    return _NC_CACHE


# ---------------- host wrapper ----------------
def make_in_maps(x, S_in, H_block, w_quantized, lora_a, lora_b, bias):
    import ml_dtypes
    BF16 = ml_dtypes.bfloat16

    x = np.asarray(x, dtype=F32)
    S_in = np.asarray(S_in, dtype=F32)
    H_block = np.asarray(H_block, dtype=F32)
    w_quantized = np.asarray(w_quantized, dtype=F32)
    lora_a = np.asarray(lora_a, dtype=F32)
    lora_b = np.asarray(lora_b, dtype=F32)
    bias = np.asarray(bias, dtype=F32)

    x_flat = x.reshape(NTOK, D_IN)
    wT = np.ascontiguousarray(w_quantized.T.astype(BF16))   # [D_IN, D_OUT]

    # sign matrix: H_block = Hpm * (1/sqrt(128)); Hpm entries are +-1 (bf16 exact)
    Hpm = np.where(H_block > 0, np.float32(1.0), np.float32(-1.0))
    Sc = S_in.reshape(NJ, HBS)                              # [j, r]
    # HmS[r, j, c] = Hpm[r, c] * S[j*128+r]
    HmS = np.ascontiguousarray(
        (Hpm[None, :, :] * Sc[:, :, None]).transpose(1, 0, 2).astype(BF16)
    )
    # la_eff[c, j, r] = S[j*128+c] * sum_k H_block[k, c] * lora_a[r, j*128+k]
    la_blk = lora_a.reshape(RANK, NJ, HBS)                  # [r, j, k]
    la_rot = np.einsum("kc,rjk->cjr", H_block, la_blk)      # [c, j, r]
    laE = np.ascontiguousarray((la_rot * Sc.T[:, :, None]).astype(BF16))
    lbTe = np.ascontiguousarray(
        np.concatenate([lora_b.T, bias.reshape(1, D_OUT)], axis=0).astype(BF16)
    )
    ident = np.ascontiguousarray(np.eye(128, dtype=F32).astype(BF16))
    sel4 = np.ascontiguousarray(
        np.tile(np.eye(RANK, dtype=F32), (4, 1)).astype(BF16)
    )

    x_hi = x_flat.astype(BF16)
    x_lo = (x_flat - x_hi.astype(F32)).astype(BF16)
    in_maps = []
    for c in range(NCORES):
        sl = slice(c * NT, (c + 1) * NT)
        # [h, q, t, jg, j, c] -> [jg, c, q, j, h, t]
        xhl = np.stack([x_hi[sl], x_lo[sl]], axis=0).reshape(
            2, NQ, QT, NJ // 4, 4, HBS
        ).transpose(3, 5, 1, 4, 0, 2)
        in_maps.append(
            {"xThl": np.ascontiguousarray(xhl),
             "wT": wT, "HmS": HmS, "laE": laE,
             "lbTe": lbTe, "ident": ident, "sel4": sel4}
        )
    return in_maps


def kernel(x, S_in, H_block, w_quantized, lora_a, lora_b, bias):
    nc = _get_nc()
    in_maps = make_in_maps(x, S_in, H_block, w_quantized, lora_a, lora_b, bias)
    res = run_bass_kernel_spmd(nc, in_maps, core_ids=list(range(NCORES)))
    out = np.concatenate([res.results[c]["y"] for c in range(NCORES)], axis=0)
    return out.reshape(B, S, D_OUT).astype(F32)


# revision 32
# speedup vs baseline: 1.1238x; 1.0047x over previous
"""CascadeHadamardLinear Trainium2 kernel (8-core data-parallel over tokens).

Math per token row x[4096]:
  x_rot = (x * S_in) @ blockdiag(H_128)
  x_q   = NVFP4 fake-quant of x_rot (16-elem blocks, e2m1 snap, RNE)
  out   = x_q @ W^T + (x_rot @ la^T) @ lb^T + bias

Host side: shard 8192 tokens over 8 cores (1024 each). x ships as a
bf16 hi/lo pair (x = hi + lo exactly to ~2^-17), the Hadamard as the
exact-in-bf16 sign matrix (H*sqrt(128))*S per block, so rotation runs
as two accumulating bf16 matmuls per tile with 1/sqrt(128) folded into
the quant scale constant. la_eff ( = per-block diag(S) H la^T ) is host
precomputed; lbT carries a 33rd row = bias matched by a ones-row in
t1T so bias needs no extra matmul.

Device, per core, tokens in 4 quarters of 256 (P1), output in 8 column
groups of 512 (P2, W loaded once, on the scalar HWDGE queue so it
doesn't sit behind P1's x loads on sync):
  P1(q) per block-group jg: rotation MMs (hi+lo) -> PSUM bank
    [128t,512] fp32; ACT evac to SBUF; quant on SBUF: absmax-16 + recip
    + x6 (DVE), z = b*rs6 (GPSIMD), two fused custom DVE snap ops
    (magic-add RNE + selects on z^2), xq = f*sc -> bf16 (GPSIMD);
    LoRA1 PSUM accumulation over j; PE-transpose xq -> xqT.
  P2 chain (og,th): PSUM accum of lora2+bias MM ([t1T;1]^T @ [lbT;bias],
    emitted first so its weight-load hides) + 32 xqT MMs, ACT evac bf16,
    DMA out. Chains for og0/og1 are injected between P1 jg steps (4 per
    quarter, matching the th tiles quantized so far) so the in-order PE
    queue stays dense while DVE/GPSIMD run the quant chain; the
    remaining column groups follow P1 with 1-deep W prefetch.
"""

import os
import sys

for _p in ("/opt/trn_rl_repo",):
    if os.path.isdir(_p) and _p not in sys.path:
        sys.path.insert(0, _p)

import numpy as np

import concourse.bass as bass
import concourse.mybir as mybir
import concourse.tile as tile
from concourse import bacc
from concourse.bass_utils import run_bass_kernel_spmd

F32 = np.float32

# ---------------- problem constants (hardcoded per contract) ----------------
B, S, D_IN, D_OUT, RANK, HBS = 4, 2048, 4096, 4096, 32, 128
NTOK = B * S                  # 8192
NCORES = 8
NT = NTOK // NCORES           # 1024 tokens per core
NJ = D_IN // HBS              # 32 hadamard blocks
QB = 16                       # quant block size
NQ = 4                        # token quarters per core
QT = NT // NQ                 # 256 tokens per quarter
OG_N = D_OUT // 512           # 8 output column groups

# 1/sqrt(128) exactly as the reference's fp32 H entries have it
H_SCALE = float(np.float32(np.float64(1.0) / np.sqrt(np.float64(128.0))))

# quant snap constants (1.5*2^k magic so ulp is uniform on both sides of c)
C_INT = 12582912.0            # 1.5*2^23, ulp 1.0
C_HALF = 6291456.0            # 1.5*2^22, ulp 0.5
TH23 = 20.25                  # 4.5^2
THF = 5.0625                  # 2.25^2

# ---------------- custom DVE ops (e2m1 level snap) ----------------
def _register_snap_ops():
    from concourse.dve_spec import (
        Spec, Src0, Src1, C0, C1, C2, lower as dve_lower, sq, select, _has_src1,
    )
    from concourse.dve_ops import (
        DveOp, OPS, CUSTOM_DVE_SPECS, _SUB_OPCODE_FOR_NAME, _CUSTOM_DVE_ROW_BASE,
    )
    from concourse.dve_uop import DveOpSpec
    from concourse.dve_table_gen import dve_ver_for

    def _ref_midlow(in0, in1, c0, c1, c2):
        z = in0.astype(F32)
        c1 = F32(c1)
        c2 = F32(c2)
        th = (z + c1) - c1
        ti = (z + c2) - c2
        return np.where(z * z < F32(c0), th, ti).astype(F32)

    def _ref_sel23(in0, in1, c0, c1, c2):
        z = in0.astype(F32)
        c3 = F32(2.0) * F32(c1)
        t3 = (z + c3) - c3
        return np.where(z * z < F32(c0), in1.astype(F32), t3).astype(F32)

    def _mk(name, body, ref):
        if name in _SUB_OPCODE_FOR_NAME:
            return next(op for op in OPS if op.name == name)
        spec = Spec(body=body, reference=ref)
        row = _CUSTOM_DVE_ROW_BASE + len(OPS)
        assert row < 0x20
        ver = dve_ver_for("TRN2")
        uops = dve_lower(spec, ver=ver)
        sha = DveOpSpec(
            name=name, opcode=row, uops=uops, rd1_en=_has_src1(spec)
        ).sha(ver)
        op = DveOp(name, spec, subdim=False, uops_sha={ver: sha})
        OPS.append(op)
        CUSTOM_DVE_SPECS[name] = spec
        _SUB_OPCODE_FOR_NAME[name] = row
        return op

    z = Src0
    midlow = _mk(
        "SNAP_MIDLOW_ANT",
        select(sq(z) < C0, (z + C1) - C1, (z + C2) - C2),
        _ref_midlow,
    )
    c3 = C1 + C1
    sel23 = _mk(
        "SNAP_SEL23_ANT",
        select(sq(z) < C0, Src1, (z + c3) - c3),
        _ref_sel23,
    )
    return midlow, sel23


SNAP_MIDLOW, SNAP_SEL23 = _register_snap_ops()


# ---------------- device kernel ----------------
def _build_nc():
    nc = bacc.Bacc(
        "TRN2", target_bir_lowering=False, debug=False, num_devices=NCORES
    )
    dt = mybir.dt
    # x pre-arranged on host: [jg, c, q, j, hi/lo, t] so each (jg, q) slice is
    # one fully-contiguous 2KB-per-partition DMA
    xThl = nc.dram_tensor(
        "xThl", [NJ // 4, HBS, NQ, 4, 2, QT], dt.bfloat16, kind="ExternalInput"
    )
    wT = nc.dram_tensor("wT", [D_IN, D_OUT], dt.bfloat16, kind="ExternalInput")
    HmS = nc.dram_tensor("HmS", [HBS, NJ, HBS], dt.bfloat16, kind="ExternalInput")
    laE = nc.dram_tensor("laE", [HBS, NJ, RANK], dt.bfloat16, kind="ExternalInput")
    lbTe = nc.dram_tensor("lbTe", [RANK + 1, D_OUT], dt.bfloat16, kind="ExternalInput")
    ident = nc.dram_tensor("ident", [128, 128], dt.bfloat16, kind="ExternalInput")
    sel4 = nc.dram_tensor("sel4", [128, RANK], dt.bfloat16, kind="ExternalInput")
    y = nc.dram_tensor("y", [NT, D_OUT], dt.bfloat16, kind="ExternalOutput")

    with tile.TileContext(nc) as tc:
        _emit(nc, tc, xThl, wT, HmS, laE, lbTe, ident, sel4, y)
    nc.compile()
    return nc


def _emit(nc, tc, xThl, wT, HmS, laE, lbTe, ident, sel4, y):
    from contextlib import ExitStack

    dt = mybir.dt
    Alu = mybir.AluOpType

    with ExitStack() as ctx:
        consts = ctx.enter_context(tc.tile_pool(name="consts", bufs=1))
        xqT_pool = ctx.enter_context(tc.tile_pool(name="xqT", bufs=1))

        # H needed by the very first rotation MM: sync queue, first.
        # Everything not needed in the first ~30us goes on the scalar queue.
        H_sb = consts.tile([HBS, NJ, HBS], dt.bfloat16)
        nc.sync.dma_start(out=H_sb[:], in_=HmS[:])
        la_sb = consts.tile([HBS, NJ, RANK], dt.bfloat16)
        nc.sync.dma_start(out=la_sb[:], in_=laE[:])
        lb_sb = consts.tile([RANK + 1, D_OUT], dt.bfloat16)
        nc.scalar.dma_start(out=lb_sb[:], in_=lbTe[:])
        id_sb = consts.tile([128, 128], dt.bfloat16)
        nc.scalar.dma_start(out=id_sb[:], in_=ident[:])
        sel4_sb = consts.tile([128, RANK], dt.bfloat16)
        nc.scalar.dma_start(out=sel4_sb[:], in_=sel4[:])
        t1T = consts.tile([RANK + 1, NT], dt.bfloat16)
        nc.vector.memset(t1T[RANK : RANK + 1, :], 1.0)

        # xqT[c, j, t] : feature-major quantized activations (bf16)
        xqT = xqT_pool.tile([128, NJ, NT], dt.bfloat16)

        xt_pool = ctx.enter_context(tc.tile_pool(name="xt", bufs=4))
        bsb_pool = ctx.enter_context(tc.tile_pool(name="bsb", bufs=4))
        qsm = ctx.enter_context(tc.tile_pool(name="qsm", bufs=12))
        qtmp = ctx.enter_context(tc.tile_pool(name="qtmp", bufs=8))
        xq_pool = ctx.enter_context(tc.tile_pool(name="xq", bufs=6))
        wbf_pool = ctx.enter_context(tc.tile_pool(name="wbf", bufs=2))
        out_pool = ctx.enter_context(tc.tile_pool(name="out", bufs=4))
        rot_ps = ctx.enter_context(tc.tile_pool(name="rotps", bufs=3, space="PSUM"))
        pt_ps = ctx.enter_context(tc.tile_pool(name="ptps", bufs=2, space="PSUM"))
        t1_ps = ctx.enter_context(tc.tile_pool(name="t1ps", bufs=1, space="PSUM"))
        out_ps = ctx.enter_context(tc.tile_pool(name="outps", bufs=2, space="PSUM"))

        def load_wbf(og):
            osl = slice(og * 512, (og + 1) * 512)
            wbf = wbf_pool.tile([128, NJ, 512], dt.bfloat16, name=f"wbf{og}", tag="wbf")
            # scalar (ACT) hwdge queue: decoupled from the xt stream on sync
            nc.scalar.dma_start(
                out=wbf[:], in_=wT[:, osl].rearrange("(j c) o -> c j o", c=HBS)
            )
            return wbf

        def emit_chain(og, th, wbf):
            osl = slice(og * 512, (og + 1) * 512)
            tsl = slice(th * 128, (th + 1) * 128)
            po = out_ps.tile([128, 512], dt.float32, name=f"po{og}_{th}", tag="po")
            # lora2+bias first: its t1T weight-load hides under the previous
            # chain's tail instead of stalling this chain's end
            nc.tensor.matmul(
                po[:], lhsT=t1T[:, tsl], rhs=lb_sb[:, osl],
                start=True, stop=False,
            )
            for j in range(NJ):
                nc.tensor.matmul(
                    po[:], lhsT=xqT[:, j, tsl], rhs=wbf[:, j, :],
                    start=False, stop=(j == NJ - 1),
                )
            ot = out_pool.tile([128, 512], dt.bfloat16, name=f"ot{og}_{th}", tag="ot")
            nc.scalar.copy(out=ot[:], in_=po[:])
            nc.scalar.dma_start(out=y[tsl, osl], in_=ot[:])

        def emit_p1(t0, tlen, inject_at):
            # token range [t0, t0+tlen) must lie within one quarter
            q, o0 = t0 // QT, t0 % QT
            osl = slice(o0, o0 + tlen)
            qsl = slice(t0, t0 + tlen)
            t1p = t1_ps.tile([RANK, tlen], dt.float32, name=f"t1p{t0}", tag="t1p")
            nts = tlen // 128

            def flush_transposes(jg, xq_tiles):
                pt = pt_ps.tile([128, 4, tlen], dt.bfloat16, name=f"pt{t0}_{jg}", tag="pt")
                for dj in range(4):
                    for ts in range(nts):
                        nc.tensor.matmul(
                            pt[:, dj, ts * 128 : (ts + 1) * 128],
                            lhsT=xq_tiles[ts][:, dj * HBS : (dj + 1) * HBS],
                            rhs=id_sb[:], is_transpose=True,
                            start=(dj == 0 and ts == 0),
                            stop=(dj == 3 and ts == nts - 1),
                        )
                nc.scalar.copy(out=xqT[:, 4 * jg : 4 * jg + 4, qsl], in_=pt[:])

            prev = None
            for jg in range(NJ // 4):
                xt = xt_pool.tile([128, 4, 2, tlen], dt.bfloat16,
                                  name=f"xt{t0}_{jg}", tag="xt")
                # two half-loads so the first j-pair lands sooner
                nc.sync.dma_start(out=xt[:, 0:2], in_=xThl[jg, :, q, 0:2, :, osl])
                nc.sync.dma_start(out=xt[:, 2:4], in_=xThl[jg, :, q, 2:4, :, osl])
                # LoRA1 accumulation over all 32 j blocks
                xq_tiles = []
                for ts in range(tlen // 128):
                    bank = rot_ps.tile([128, 512], dt.float32,
                                       name=f"bank{t0}_{jg}{ts}", tag="bank")
                    for dj in range(4):
                        j = 4 * jg + dj
                        for hl in range(2):
                            nc.tensor.matmul(
                                bank[:, dj * HBS : (dj + 1) * HBS],
                                lhsT=xt[:, dj, hl, ts * 128 : (ts + 1) * 128],
                                rhs=H_sb[:, j, :],
                                start=(dj == 0 and hl == 0),
                                stop=(dj == 3 and hl == 1),
                            )
                    bsb = bsb_pool.tile([128, 512], dt.float32,
                                        name=f"bsb{t0}_{jg}{ts}", tag="bsb")
                    nc.scalar.copy(out=bsb[:], in_=bank[:])
                    nb = 512 // QB
                    amax = qsm.tile([128, nb], dt.float32, name=f"amax{t0}_{jg}{ts}", tag="amax")
                    nc.vector.tensor_reduce(
                        out=amax[:], in_=bsb[:].rearrange("p (b s) -> p b s", s=QB),
                        axis=mybir.AxisListType.X, op=Alu.max,
                        apply_absolute_value=True,
                    )
                    ra = qsm.tile([128, nb], dt.float32, name=f"ra{t0}_{jg}{ts}", tag="ra")
                    nc.vector.reciprocal(out=ra[:], in_=amax[:])
                    rs6 = qsm.tile([128, nb], dt.float32, name=f"rs6{t0}_{jg}{ts}", tag="rs6")
                    nc.vector.tensor_scalar(
                        out=rs6[:], in0=ra[:], scalar1=6.0, scalar2=None,
                        op0=Alu.mult,
                    )
                    sc = qsm.tile([128, nb], dt.float32, name=f"sc{t0}_{jg}{ts}", tag="sc")
                    nc.scalar.mul(out=sc[:], in_=amax[:], mul=H_SCALE / 6.0)
                    z = qtmp.tile([128, 512], dt.float32, name=f"z{t0}_{jg}{ts}", tag="qt")
                    nc.gpsimd.tensor_tensor(
                        out=z[:].rearrange("p (b s) -> p b s", s=QB),
                        in0=bsb[:].rearrange("p (b s) -> p b s", s=QB),
                        in1=rs6[:].unsqueeze(2).broadcast_to([128, nb, QB]),
                        op=Alu.mult,
                    )
                    r = qtmp.tile([128, 512], dt.float32, name=f"r{t0}_{jg}{ts}", tag="qt")
                    nc.vector._custom_dve(
                        SNAP_MIDLOW, out=r[:], in0=z[:], s0=THF, s1=C_HALF,
                        imm2=C_INT,
                    )
                    f = qtmp.tile([128, 512], dt.float32, name=f"f{t0}_{jg}{ts}", tag="qt")
                    nc.vector._custom_dve(
                        SNAP_SEL23, out=f[:], in0=z[:], in1=r[:], s0=TH23, s1=C_INT,
                    )
                    xq_t = xq_pool.tile([128, 512], dt.bfloat16, name=f"xq{t0}_{jg}{ts}", tag="xq")
                    nc.gpsimd.tensor_tensor(
                        out=xq_t[:].rearrange("p (b s) -> p b s", s=QB),
                        in0=f[:].rearrange("p (b s) -> p b s", s=QB),
                        in1=sc[:].unsqueeze(2).broadcast_to([128, nb, QB]),
                        op=Alu.mult,
                    )
                    xq_tiles.append(xq_t)
                # LoRA1 after the rotation MMs: the first PE op of the kernel
                # then needs only H and x, la loads in parallel
                for dj in range(4):
                    j = 4 * jg + dj
                    nc.tensor.matmul(
                        t1p[:], lhsT=la_sb[:, j, :], rhs=xt[:, dj, 0, :],
                        start=(j == 0), stop=(j == NJ - 1),
                    )
                # transposes run one jg step late: by the time they reach
                # the head of the in-order PE queue their quant inputs are
                # done, so this jg's rotation and an injected P2 chain fill
                # the PE instead of stalling on the quant chain
                if prev is not None:
                    flush_transposes(*prev)
                if jg in inject_at:
                    inject()
                prev = (jg, xq_tiles)
            flush_transposes(*prev)
            nc.scalar.copy(out=t1T[0:RANK, qsl], in_=t1p[:])

        # P2 chains for og0/og1 injected into P1; W preloaded up front
        wbf_pre = [load_wbf(0), load_wbf(1)]
        pending = [(og, th) for th in range(NT // 128) for og in (0, 1)]
        pos = 0

        def inject():
            nonlocal pos
            if pos < len(pending):
                og, th = pending[pos]
                pos += 1
                emit_chain(og, th, wbf_pre[og])

        # four 256-token quarters; chains for already-quantized token tiles
        # are injected mid-quarter (pending is th-major so pops always match
        # available th tiles: q1 injects th0/th1, q2 th2/th3, q3 th4/th5)
        emit_p1(0, QT, ())
        emit_p1(QT, QT, (1, 3, 5, 7))
        emit_p1(2 * QT, QT, (1, 3, 5, 7))
        emit_p1(3 * QT, QT, (1, 3, 5, 7))
        # leftovers: og0 first so its wbf slot frees for og2's prefetch,
        # og1's leftovers then hide og2's transfer
        left = [pt for pt in pending[pos:] if pt[0] == 0] + \
               [pt for pt in pending[pos:] if pt[0] == 1]
        mid = len([pt for pt in left if pt[0] == 0])
        for i, (og, th) in enumerate(left):
            if i == mid:
                wbf_next = load_wbf(2)
            emit_chain(og, th, wbf_pre[og])
        if mid == len(left):
            wbf_next = load_wbf(2)
        for og in range(2, OG_N):
            wbf = wbf_next
            wbf_next = load_wbf(og + 1) if og + 1 < OG_N else None
            for th in range(NT // 128):
                emit_chain(og, th, wbf)


_NC_CACHE = None


def _get_nc():
    global _NC_CACHE
    if _NC_CACHE is None:
        _NC_CACHE = _build_nc()
    return _NC_CACHE


# ---------------- host wrapper ----------------
def make_in_maps(x, S_in, H_block, w_quantized, lora_a, lora_b, bias):
    import ml_dtypes
    BF16 = ml_dtypes.bfloat16

    x = np.asarray(x, dtype=F32)
    S_in = np.asarray(S_in, dtype=F32)
    H_block = np.asarray(H_block, dtype=F32)
    w_quantized = np.asarray(w_quantized, dtype=F32)
    lora_a = np.asarray(lora_a, dtype=F32)
    lora_b = np.asarray(lora_b, dtype=F32)
    bias = np.asarray(bias, dtype=F32)

    x_flat = x.reshape(NTOK, D_IN)
    wT = np.ascontiguousarray(w_quantized.T.astype(BF16))   # [D_IN, D_OUT]

    # sign matrix: H_block = Hpm * (1/sqrt(128)); Hpm entries are +-1 (bf16 exact)
    Hpm = np.where(H_block > 0, np.float32(1.0), np.float32(-1.0))
    Sc = S_in.reshape(NJ, HBS)                              # [j, r]
    # HmS[r, j, c] = Hpm[r, c] * S[j*128+r]
    HmS = np.ascontiguousarray(
        (Hpm[None, :, :] * Sc[:, :, None]).transpose(1, 0, 2).astype(BF16)
    )
    # la_eff[c, j, r] = S[j*128+c] * sum_k H_block[k, c] * lora_a[r, j*128+k]
    la_blk = lora_a.reshape(RANK, NJ, HBS)                  # [r, j, k]
    la_rot = np.einsum("kc,rjk->cjr", H_block, la_blk)      # [c, j, r]
    laE = np.ascontiguousarray((la_rot * Sc.T[:, :, None]).astype(BF16))
    lbTe = np.ascontiguousarray(
        np.concatenate([lora_b.T, bias.reshape(1, D_OUT)], axis=0).astype(BF16)
    )
    ident = np.ascontiguousarray(np.eye(128, dtype=F32).astype(BF16))
    sel4 = np.ascontiguousarray(
        np.tile(np.eye(RANK, dtype=F32), (4, 1)).astype(BF16)
    )

    x_hi = x_flat.astype(BF16)
    x_lo = (x_flat - x_hi.astype(F32)).astype(BF16)
    in_maps = []
    for c in range(NCORES):
        sl = slice(c * NT, (c + 1) * NT)
        # [h, q, t, jg, j, c] -> [jg, c, q, j, h, t]
        xhl = np.stack([x_hi[sl], x_lo[sl]], axis=0).reshape(
            2, NQ, QT, NJ // 4, 4, HBS
        ).transpose(3, 5, 1, 4, 0, 2)
        in_maps.append(
            {"xThl": np.ascontiguousarray(xhl),
             "wT": wT, "HmS": HmS, "laE": laE,
             "lbTe": lbTe, "ident": ident, "sel4": sel4}
        )
    return in_maps


def kernel(x, S_in, H_block, w_quantized, lora_a, lora_b, bias):
    nc = _get_nc()
    in_maps = make_in_maps(x, S_in, H_block, w_quantized, lora_a, lora_b, bias)
    res = run_bass_kernel_spmd(nc, in_maps, core_ids=list(range(NCORES)))
    out = np.concatenate([res.results[c]["y"] for c in range(NCORES)], axis=0)
    return out.reshape(B, S, D_OUT).astype(F32)
